# revision 9
# baseline (speedup 1.0000x reference)
"""Expert-parallel MoE FFN for Trainium2 — one expert per NeuronCore (8 cores).

Strategy
--------
The reference computes, per token, the sum of top-2 expert FFN outputs (binary
combine mask, no gate weighting).  We shard along the expert axis: core ``e``
holds expert ``e``'s weights (W1[e], b1[e], W2[e], b2[e]) and processes only
the tokens that routed to it.

Host side (cheap, O(T*D*E) = 34 MFLOP):
  * gating softmax + top-2 (replicates jax.nn.softmax + jax.lax.top_k
    tie-breaking exactly: stable argsort on the fp32 scores, descending),
  * gather each expert's tokens, pad to a uniform capacity (all cores run the
    same NEFF), pre-transpose AND pre-pack every tensor into its exact SBUF
    layout ([128 partitions, flat free dim]) so each device DMA is a single
    trigger moving full-row (multi-KB) packets,
  * scatter-add the 8 per-expert outputs back into the [T, D] result.

Device side (the heavy part, ~18 GFLOP/core):
  hT = relu(W1^T-chained matmuls + b1);  yT = W2-chained matmuls + b2,
  everything kept in "transposed" layout: contraction dims live on SBUF
  partitions for both layers, so mm1's output feeds mm2 directly.
  bf16 inputs, fp32 PSUM accumulation, bf16 output.

Schedule (v2) — the warm matmul stream runs at the issue floor (~N/2.4GHz +
2.6ns), so the only recoverable time is at the edges:
  * ~56 zero-input warmup matmuls issue right after the engine preamble with
    no DMA dependency: they warm the PE HAM clock-gate (cold K=4/8 costs 2x)
    and absorb the DMA spin-up so real matmuls start warm and stall-free.
  * x token tiles ride the Sync/HWDGE queue concurrently with the W1 chain on
    the GpSimd/SWDGE queue (the two queues share the 16 DMA engines; measured
    aggregate ~360 GB/s).
  * W1 is split into ascending-size f-column groups so the first group lands
    fast; W2 is packed m-major (8 groups of [128, KF*128]) so groups complete
    in exactly mm2's m-loop consumption order.
  * y is emitted bf16 (halves the tail DMA; adds ~0.2% rms, well inside the
    2e-2 budget).
"""

import numpy as np
import ml_dtypes

import concourse.bacc as bacc
import concourse.mybir as mybir
import concourse.tile as tile
from concourse.bass_utils import run_bass_kernel_spmd
from concourse._compat import get_trn_type

D_MODEL = 1024
D_FF = 4096
N_EXP = 8
TOP_K = 2
KD = D_MODEL // 128  # 8 contraction chunks over d_model
KF = D_FF // 128  # 32 contraction chunks over d_ff

# W1 f-column groups: fine 128-col singles up front (whole-tile DMA deps —
# smaller groups complete sooner, so mm1's f-loop never outruns the ring),
# coarser groups once supply is comfortably ahead of consumption.
W1_GROUPS = [
    (0, 128), (128, 256), (256, 384), (384, 512),
    (512, 640), (640, 768), (768, 896), (896, 1024),
    (1024, 1536), (1536, 2176), (2176, 2944), (2944, 3712), (3712, D_FF),
]
N_W1_EARLY = 4  # first groups ride the scalar queue (earliest preamble exit)

# Zero-input warmup matmuls (N=128): ~107ns each cold; they bridge the gap
# between engine-preamble end (~6.6us) and the first operands landing
# (~9.7us) while warming the HAM clock gate (cold K=4/8 halves PE clock
# for ~3.4us after first activity).
N_WARM = 32

BF16 = mybir.dt.bfloat16
F32 = mybir.dt.float32

_programs: dict[tuple, object] = {}


def _build_program(cap: int, tt: int):
    """Bass/Tile program: pre-packed [D,cap] tokens -> 2-layer FFN -> output."""
    assert cap % tt == 0
    nt = cap // tt
    nc = bacc.Bacc(get_trn_type() or "TRN2", target_bir_lowering=False, debug=False)

    # x tile 0 in three k-chunks (k0-1, k2-4, k5-7) + the rest.
    xg_names = ["x0a", "x0b", "x0c"] + (["x1"] if nt > 1 else [])
    xg_widths = [2 * tt, 3 * tt, 3 * tt] + ([KD * (cap - tt)] if nt > 1 else [])
    xg_d = {
        n: nc.dram_tensor(n, [128, w], BF16, kind="ExternalInput").ap()
        for n, w in zip(xg_names, xg_widths)
    }
    w1_d = [
        nc.dram_tensor(f"W1{g}", [128, KD * (hi - lo)], BF16, kind="ExternalInput").ap()
        for g, (lo, hi) in enumerate(W1_GROUPS)
    ]
    # W2 m-major: group m holds [128 (f-part), KF * 128 m-cols].
    w2_d = [
        nc.dram_tensor(f"W2m{m}", [128, KF * 128], BF16, kind="ExternalInput").ap()
        for m in range(KD)
    ]
    b1_d = nc.dram_tensor("b1", [128, KF], F32, kind="ExternalInput").ap()
    b2_d = nc.dram_tensor("b2", [128, KD], F32, kind="ExternalInput").ap()
    y_d = nc.dram_tensor("yT", [128, KD * cap], BF16, kind="ExternalOutput").ap()
    y_v = y_d.rearrange("p (m c) -> p m c", c=cap)

    with tile.TileContext(nc) as tc:
        with (
            tc.tile_pool(name="sb", bufs=1) as sb,
            tc.tile_pool(name="hp", bufs=40) as hp,
            tc.tile_pool(name="yp", bufs=4) as yp,
            tc.tile_pool(name="pp1", bufs=6, space="PSUM") as pp1,
            tc.tile_pool(name="pp2", bufs=2, space="PSUM") as pp2,
        ):
            # ---- tiles ---------------------------------------------------
            x_sb = {
                n: sb.tile([128, d.shape[1]], BF16, tag=n, name=f"{n}_sb")
                for n, d in xg_d.items()
            }
            w1_tiles = [
                sb.tile([128, KD * (hi - lo)], BF16, tag=f"w1g{g}", name=f"w1g{g}")
                for g, (lo, hi) in enumerate(W1_GROUPS)
            ]
            w1_gs = [(lo, hi, t) for (lo, hi), t in zip(W1_GROUPS, w1_tiles)]
            b1_sb = sb.tile([128, KF], F32, tag="b1", name="b1_sb")
            b2_sb = sb.tile([128, KD], F32, tag="b2", name="b2_sb")
            w2_tiles = [
                sb.tile([128, KF * 128], BF16, tag=f"w2m{m}", name=f"w2m{m}")
                for m in range(KD)
            ]
            z_sb = sb.tile([128, 128], BF16, tag="zw", name="zw")

            # ---- input triggers + PE warmup ------------------------------
            # Scalar queue (earliest preamble exit): b1 + first W1 singles.
            nc.scalar.dma_start(b1_sb[:], b1_d)
            for g in range(N_W1_EARLY):
                nc.scalar.dma_start(w1_tiles[g][:], w1_d[g])
            # Sync queue: x tile-0 chunks.
            nc.sync.dma_start(x_sb["x0a"][:], xg_d["x0a"])
            nc.vector.memset(z_sb[:], 0.0)
            nc.sync.dma_start(x_sb["x0b"][:], xg_d["x0b"])
            nc.sync.dma_start(x_sb["x0c"][:], xg_d["x0c"])
            # GpSimd queue: remaining weights in consumption order.
            for g in range(N_W1_EARLY, len(W1_GROUPS)):
                nc.gpsimd.dma_start(w1_tiles[g][:], w1_d[g])
            nc.gpsimd.dma_start(b2_sb[:], b2_d)
            for m in range(KD):
                nc.gpsimd.dma_start(w2_tiles[m][:], w2_d[m])
            if nt > 1:
                nc.gpsimd.dma_start(x_sb["x1"][:], xg_d["x1"])

            # Zero matmuls with no DMA dependency: keep the PE busy (and the
            # HAM clock-gate warming) while the first operands land.
            wps = pp2.tile([128, 128], F32, tag="ps2", name="warm_ps")
            for _ in range(N_WARM):
                nc.tensor.matmul(wps[:], z_sb[:], z_sb[:], start=True, stop=True)

            def x_rhs(k, it):
                if it == 0:
                    if k < 2:
                        return x_sb["x0a"][:, k * tt : (k + 1) * tt]
                    if k < 5:
                        return x_sb["x0b"][:, (k - 2) * tt : (k - 1) * tt]
                    return x_sb["x0c"][:, (k - 5) * tt : (k - 4) * tt]
                rest = cap - tt
                lo = k * rest + (it - 1) * tt
                return x_sb["x1"][:, lo : lo + tt]

            def w1_lhsT(k, f):
                col = f * 128
                for lo, hi, t in w1_gs:
                    if lo <= col < hi:
                        base = k * (hi - lo) + (col - lo)
                        return t[:, base : base + 128]
                raise AssertionError

            def w2_lhsT(f, m):
                return w2_tiles[m][:, f * 128 : (f + 1) * 128]

            # ---- compute --------------------------------------------------
            for it in range(nt):
                # mm1: hT[f*128+p, t] = relu(sum_d W1[d, f*128+p]*xT[d, t] + b1)
                h_tiles = []
                for f in range(KF):
                    ps = pp1.tile([128, tt], F32, tag="ps1", name=f"ps1_{it}_{f}")
                    for k in range(KD):
                        nc.tensor.matmul(
                            ps[:],
                            w1_lhsT(k, f),
                            x_rhs(k, it),
                            start=(k == 0),
                            stop=(k == KD - 1),
                        )
                    ht = hp.tile([128, tt], BF16, tag="h", name=f"h_{it}_{f}")
                    nc.scalar.activation(
                        ht[:],
                        ps[:],
                        mybir.ActivationFunctionType.Relu,
                        bias=b1_sb[:, f : f + 1],
                    )
                    h_tiles.append(ht)

                # mm2: yT[m*128+p, t] = sum_f W2[f, m*128+p] * hT[f, t] + b2
                for m in range(KD):
                    ps2 = pp2.tile([128, tt], F32, tag="ps2", name=f"ps2_{it}_{m}")
                    for f in range(KF):
                        nc.tensor.matmul(
                            ps2[:],
                            w2_lhsT(f, m),
                            h_tiles[f][:],
                            start=(f == 0),
                            stop=(f == KF - 1),
                        )
                    last = it == nt - 1 and m == KD - 1
                    yt = yp.tile([128, tt], BF16, tag="y", name=f"y_{it}_{m}")
                    if not last:
                        nc.vector.tensor_scalar_add(
                            yt[:], ps2[:], b2_sb[:, m : m + 1]
                        )
                        nc.sync.dma_start(y_v[:, m, it * tt : (it + 1) * tt], yt[:])
                    else:
                        # Critical tail: split the final chunk across two
                        # engine pairs so add+trigger+transfer pipeline.
                        th = tt // 2
                        nc.scalar.activation(
                            yt[:, :th],
                            ps2[:, :th],
                            mybir.ActivationFunctionType.Identity,
                            bias=b2_sb[:, m : m + 1],
                        )
                        nc.sync.dma_start(
                            y_v[:, m, it * tt : it * tt + th], yt[:, :th]
                        )
                        nc.vector.tensor_scalar_add(
                            yt[:, th:], ps2[:, th:], b2_sb[:, m : m + 1]
                        )
                        nc.gpsimd.dma_start(
                            y_v[:, m, it * tt + th : (it + 1) * tt], yt[:, th:]
                        )

    nc.compile()
    return nc


def _gating_topk(x, Wg, bg):
    """Replicates jax.nn.softmax + jax.lax.top_k(..., 2) in fp32 numpy."""
    logits = x @ Wg + bg
    m = logits.max(axis=1, keepdims=True)
    e = np.exp(logits - m)
    scores = e / e.sum(axis=1, keepdims=True)
    # top_k: descending, ties broken toward the lower index (stable).
    order = np.argsort(-scores, axis=1, kind="stable")
    return order[:, :TOP_K]


def _capacity(max_count):
    # Token tile <= 384: keeps one fp32 PSUM bank per matmul (<=512) AND the
    # resident-weights SBUF budget valid for capacities well beyond the
    # ~1024+-27 expert loads this distribution produces.
    nt = max(1, -(-max_count // 384))
    tt = -(-max_count // nt)
    tt = -(-tt // 4) * 4  # multiple of 4 for aligned fp32 rows
    return nt * tt, tt


def _pack_k128(a):
    """[K*128, F] -> [128, K*F]: partition-major packing of the SBUF layout."""
    k128, f = a.shape
    return np.ascontiguousarray(
        a.reshape(k128 // 128, 128, f).transpose(1, 0, 2).reshape(128, -1)
    )


def _prepare(x, Wg, bg, W1, b1, W2, b2):
    x = np.ascontiguousarray(np.asarray(x, dtype=np.float32))
    topk = _gating_topk(x, np.asarray(Wg, np.float32), np.asarray(bg, np.float32))
    idx = [np.nonzero((topk == e).any(axis=1))[0] for e in range(N_EXP)]
    counts = [len(i) for i in idx]
    cap, tt = _capacity(max(counts))
    nt = cap // tt

    bf16 = ml_dtypes.bfloat16
    in_maps = []
    for e in range(N_EXP):
        xg = np.zeros((cap, D_MODEL), np.float32)
        xg[: counts[e]] = x[idx[e]]
        xT = np.ascontiguousarray(xg.T).astype(bf16)  # [D, cap]
        xTp = _pack_k128(xT).reshape(128, KD, cap)  # [128, k, c]
        w1 = np.asarray(W1[e], np.float32).astype(bf16)  # [D, DFF]
        w1p = _pack_k128(w1).reshape(128, KD, D_FF)  # [128, k, f]
        w2 = np.asarray(W2[e], np.float32).astype(bf16)  # [DFF, D]
        w2p = _pack_k128(w2).reshape(128, KF, D_MODEL)  # [128, f, m]
        m = {
            "x0a": np.ascontiguousarray(xTp[:, :2, :tt]).reshape(128, -1),
            "x0b": np.ascontiguousarray(xTp[:, 2:5, :tt]).reshape(128, -1),
            "x0c": np.ascontiguousarray(xTp[:, 5:, :tt]).reshape(128, -1),
            "b1": np.ascontiguousarray(
                np.asarray(b1[e], np.float32).reshape(KF, 128).T
            ),
            "b2": np.ascontiguousarray(
                np.asarray(b2[e], np.float32).reshape(KD, 128).T
            ),
        }
        if nt > 1:
            m["x1"] = np.ascontiguousarray(xTp[:, :, tt:]).reshape(128, -1)
        for g, (lo, hi) in enumerate(W1_GROUPS):
            m[f"W1{g}"] = np.ascontiguousarray(w1p[:, :, lo:hi]).reshape(128, -1)
        for mi in range(KD):
            m[f"W2m{mi}"] = np.ascontiguousarray(
                w2p[:, :, mi * 128 : (mi + 1) * 128]
            ).reshape(128, -1)
        in_maps.append(m)
    return x, idx, counts, cap, tt, in_maps


def _run(x, Wg, bg, W1, b1, W2, b2, **run_kwargs):
    x, idx, counts, cap, tt, in_maps = _prepare(x, Wg, bg, W1, b1, W2, b2)
    key = (cap, tt)
    prog = _programs.get(key)
    if prog is None:
        prog = _programs.setdefault(key, _build_program(cap, tt))
    res = run_bass_kernel_spmd(
        prog, in_maps, core_ids=list(range(N_EXP)), **run_kwargs
    )
    out = np.zeros_like(x)
    for e in range(N_EXP):
        yp = np.asarray(res.results[e]["yT"], np.float32)  # [128, KD*cap]
        yT = yp.reshape(128, KD, cap).transpose(1, 0, 2).reshape(D_MODEL, cap)
        out[idx[e]] += yT[:, : counts[e]].T
    return out, res


def kernel(x, Wg, bg, W1, b1, W2, b2):
    out, _ = _run(x, Wg, bg, W1, b1, W2, b2)
    return out


# revision 12
# speedup vs baseline: 1.0422x; 1.0422x over previous
"""Expert-parallel MoE FFN for Trainium2 — one expert per NeuronCore (8 cores).

Strategy
--------
The reference computes, per token, the sum of top-2 expert FFN outputs (binary
combine mask, no gate weighting).  We shard along the expert axis: core ``e``
holds expert ``e``'s weights (W1[e], b1[e], W2[e], b2[e]) and processes only
the tokens that routed to it.

Host side (cheap, O(T*D*E) = 34 MFLOP):
  * gating softmax + top-2 (replicates jax.nn.softmax + jax.lax.top_k
    tie-breaking exactly: stable argsort on the fp32 scores, descending),
  * gather each expert's tokens, pad to a uniform capacity (all cores run the
    same NEFF), pre-transpose AND pre-pack every tensor into its exact SBUF
    layout ([128 partitions, flat free dim]) so each device DMA is a single
    trigger moving full-row (multi-KB) packets,
  * scatter-add the 8 per-expert outputs back into the [T, D] result.

Device side (the heavy part, ~18 GFLOP/core):
  hT = relu(W1^T-chained matmuls + b1);  yT = W2-chained matmuls + b2,
  everything kept in "transposed" layout: contraction dims live on SBUF
  partitions for both layers, so mm1's output feeds mm2 directly.
  bf16 inputs, fp32 PSUM accumulation, bf16 output.

Schedule (v2) — the warm matmul stream runs at the issue floor (~N/2.4GHz +
2.6ns), so the only recoverable time is at the edges:
  * ~56 zero-input warmup matmuls issue right after the engine preamble with
    no DMA dependency: they warm the PE HAM clock-gate (cold K=4/8 costs 2x)
    and absorb the DMA spin-up so real matmuls start warm and stall-free.
  * x token tiles ride the Sync/HWDGE queue concurrently with the W1 chain on
    the GpSimd/SWDGE queue (the two queues share the 16 DMA engines; measured
    aggregate ~360 GB/s).
  * W1 is split into ascending-size f-column groups so the first group lands
    fast; W2 is packed m-major (8 groups of [128, KF*128]) so groups complete
    in exactly mm2's m-loop consumption order.
  * y is emitted bf16 (halves the tail DMA; adds ~0.2% rms, well inside the
    2e-2 budget).
"""

import numpy as np
import ml_dtypes

import concourse.bacc as bacc
import concourse.mybir as mybir
import concourse.tile as tile
from concourse.bass_utils import run_bass_kernel_spmd
from concourse._compat import get_trn_type

D_MODEL = 1024
D_FF = 4096
N_EXP = 8
TOP_K = 2
KD = D_MODEL // 128  # 8 contraction chunks over d_model
KF = D_FF // 128  # 32 contraction chunks over d_ff

# W1 f-column groups: fine 128-col singles up front (whole-tile DMA deps —
# smaller groups complete sooner, so mm1's f-loop never outruns the ring),
# coarser groups once supply is comfortably ahead of consumption.
W1_GROUPS = [
    (0, 128), (128, 256), (256, 384), (384, 512),
    (512, 640), (640, 768), (768, 896), (896, 1024),
    (1024, 1536), (1536, 2176), (2176, 2944), (2944, 3712), (3712, D_FF),
]

# Zero-input warmup matmuls (N=128): ~107ns each cold; they bridge the gap
# between engine-preamble end (~6.6us) and the first operands landing
# (~11us) while warming the HAM clock gate (cold K=4/8 halves PE clock for
# ~3.4us after first activity; any PE idle gap >3.4us re-throttles).
N_WARM = 40

BF16 = mybir.dt.bfloat16
F32 = mybir.dt.float32

_programs: dict[tuple, object] = {}


def _build_program(cap: int, tt: int):
    """Bass/Tile program: pre-packed [D,cap] tokens -> 2-layer FFN -> output."""
    assert cap % tt == 0
    nt = cap // tt
    nc = bacc.Bacc(get_trn_type() or "TRN2", target_bir_lowering=False, debug=False)

    # x tile 0 in three k-chunks (k0-1, k2-4, k5-7) + the rest.
    xg_names = ["x0a", "x0b", "x0c"] + (["x1"] if nt > 1 else [])
    xg_widths = [2 * tt, 3 * tt, 3 * tt] + ([KD * (cap - tt)] if nt > 1 else [])
    xg_d = {
        n: nc.dram_tensor(n, [128, w], BF16, kind="ExternalInput").ap()
        for n, w in zip(xg_names, xg_widths)
    }
    w1_d = [
        nc.dram_tensor(f"W1{g}", [128, KD * (hi - lo)], BF16, kind="ExternalInput").ap()
        for g, (lo, hi) in enumerate(W1_GROUPS)
    ]
    # W2 m-major: group m holds [128 (f-part), KF * 128 m-cols].
    w2_d = [
        nc.dram_tensor(f"W2m{m}", [128, KF * 128], BF16, kind="ExternalInput").ap()
        for m in range(KD)
    ]
    b1_d = nc.dram_tensor("b1", [128, KF], F32, kind="ExternalInput").ap()
    b2_d = nc.dram_tensor("b2", [128, KD], F32, kind="ExternalInput").ap()
    y_d = nc.dram_tensor("yT", [128, KD * cap], BF16, kind="ExternalOutput").ap()
    y_v = y_d.rearrange("p (m c) -> p m c", c=cap)

    with tile.TileContext(nc) as tc:
        with (
            tc.tile_pool(name="sb", bufs=1) as sb,
            tc.tile_pool(name="hp", bufs=40) as hp,
            tc.tile_pool(name="yp", bufs=4) as yp,
            tc.tile_pool(name="pp1", bufs=6, space="PSUM") as pp1,
            tc.tile_pool(name="pp2", bufs=2, space="PSUM") as pp2,
        ):
            # ---- tiles ---------------------------------------------------
            x_sb = {
                n: sb.tile([128, d.shape[1]], BF16, tag=n, name=f"{n}_sb")
                for n, d in xg_d.items()
            }
            w1_tiles = [
                sb.tile([128, KD * (hi - lo)], BF16, tag=f"w1g{g}", name=f"w1g{g}")
                for g, (lo, hi) in enumerate(W1_GROUPS)
            ]
            w1_gs = [(lo, hi, t) for (lo, hi), t in zip(W1_GROUPS, w1_tiles)]
            b1_sb = sb.tile([128, KF], F32, tag="b1", name="b1_sb")
            b2_sb = sb.tile([128, KD], F32, tag="b2", name="b2_sb")
            w2_tiles = [
                sb.tile([128, KF * 128], BF16, tag=f"w2m{m}", name=f"w2m{m}")
                for m in range(KD)
            ]
            z_sb = sb.tile([128, 128], BF16, tag="zw", name="zw")

            # ---- input triggers + PE warmup ------------------------------
            # ALL inputs ride ONE queue (scalar — earliest preamble exit) in
            # exact consumption order: the 16-engine DMA pool round-robins
            # across active queues, so a second queue would steal half the
            # bandwidth from the critical prefix (measured: 3 active queues
            # cut the x+W1 prefix to ~95 GB/s and stalled the PE).
            nc.vector.memset(z_sb[:], 0.0)
            nc.scalar.dma_start(b1_sb[:], b1_d)
            nc.scalar.dma_start(x_sb["x0a"][:], xg_d["x0a"])
            nc.scalar.dma_start(w1_tiles[0][:], w1_d[0])
            nc.scalar.dma_start(x_sb["x0b"][:], xg_d["x0b"])
            nc.scalar.dma_start(x_sb["x0c"][:], xg_d["x0c"])
            for g in range(1, len(W1_GROUPS)):
                nc.scalar.dma_start(w1_tiles[g][:], w1_d[g])
            nc.scalar.dma_start(b2_sb[:], b2_d)
            for m in range(KD):
                nc.scalar.dma_start(w2_tiles[m][:], w2_d[m])
            if nt > 1:
                nc.scalar.dma_start(x_sb["x1"][:], xg_d["x1"])

            # Zero matmuls with no DMA dependency: keep the PE busy (and the
            # HAM clock-gate warming) while the first operands land.
            wps = pp2.tile([128, 128], F32, tag="ps2", name="warm_ps")
            for _ in range(N_WARM):
                nc.tensor.matmul(wps[:], z_sb[:], z_sb[:], start=True, stop=True)

            def x_rhs(k, it):
                if it == 0:
                    if k < 2:
                        return x_sb["x0a"][:, k * tt : (k + 1) * tt]
                    if k < 5:
                        return x_sb["x0b"][:, (k - 2) * tt : (k - 1) * tt]
                    return x_sb["x0c"][:, (k - 5) * tt : (k - 4) * tt]
                rest = cap - tt
                lo = k * rest + (it - 1) * tt
                return x_sb["x1"][:, lo : lo + tt]

            def w1_lhsT(k, f):
                col = f * 128
                for lo, hi, t in w1_gs:
                    if lo <= col < hi:
                        base = k * (hi - lo) + (col - lo)
                        return t[:, base : base + 128]
                raise AssertionError

            def w2_lhsT(f, m):
                return w2_tiles[m][:, f * 128 : (f + 1) * 128]

            # ---- compute --------------------------------------------------
            for it in range(nt):
                # mm1: hT[f*128+p, t] = relu(sum_d W1[d, f*128+p]*xT[d, t] + b1)
                h_tiles = []
                for f in range(KF):
                    ps = pp1.tile([128, tt], F32, tag="ps1", name=f"ps1_{it}_{f}")
                    for k in range(KD):
                        nc.tensor.matmul(
                            ps[:],
                            w1_lhsT(k, f),
                            x_rhs(k, it),
                            start=(k == 0),
                            stop=(k == KD - 1),
                        )
                    # relu on the VECTOR engine: the scalar engine spends the
                    # first ~23us issuing the 26 serialized DMA triggers.
                    ht = hp.tile([128, tt], BF16, tag="h", name=f"h_{it}_{f}")
                    nc.vector.tensor_scalar(
                        ht[:],
                        ps[:],
                        b1_sb[:, f : f + 1],
                        0.0,
                        mybir.AluOpType.add,
                        mybir.AluOpType.max,
                    )
                    h_tiles.append(ht)

                # mm2: yT[m*128+p, t] = sum_f W2[f, m*128+p] * hT[f, t] + b2
                for m in range(KD):
                    ps2 = pp2.tile([128, tt], F32, tag="ps2", name=f"ps2_{it}_{m}")
                    for f in range(KF):
                        nc.tensor.matmul(
                            ps2[:],
                            w2_lhsT(f, m),
                            h_tiles[f][:],
                            start=(f == 0),
                            stop=(f == KF - 1),
                        )
                    last = it == nt - 1 and m == KD - 1
                    yt = yp.tile([128, tt], BF16, tag="y", name=f"y_{it}_{m}")
                    if not last:
                        nc.vector.tensor_scalar_add(
                            yt[:], ps2[:], b2_sb[:, m : m + 1]
                        )
                        nc.sync.dma_start(y_v[:, m, it * tt : (it + 1) * tt], yt[:])
                    else:
                        # Critical tail: split the final chunk across two
                        # engine pairs so add+trigger+transfer pipeline.
                        th = tt // 2
                        nc.scalar.activation(
                            yt[:, :th],
                            ps2[:, :th],
                            mybir.ActivationFunctionType.Identity,
                            bias=b2_sb[:, m : m + 1],
                        )
                        nc.sync.dma_start(
                            y_v[:, m, it * tt : it * tt + th], yt[:, :th]
                        )
                        nc.vector.tensor_scalar_add(
                            yt[:, th:], ps2[:, th:], b2_sb[:, m : m + 1]
                        )
                        nc.gpsimd.dma_start(
                            y_v[:, m, it * tt + th : (it + 1) * tt], yt[:, th:]
                        )

    nc.compile()
    return nc


def _gating_topk(x, Wg, bg):
    """Replicates jax.nn.softmax + jax.lax.top_k(..., 2) in fp32 numpy."""
    logits = x @ Wg + bg
    m = logits.max(axis=1, keepdims=True)
    e = np.exp(logits - m)
    scores = e / e.sum(axis=1, keepdims=True)
    # top_k: descending, ties broken toward the lower index (stable).
    order = np.argsort(-scores, axis=1, kind="stable")
    return order[:, :TOP_K]


def _capacity(max_count):
    # Token tile <= 384: keeps one fp32 PSUM bank per matmul (<=512) AND the
    # resident-weights SBUF budget valid for capacities well beyond the
    # ~1024+-27 expert loads this distribution produces.
    nt = max(1, -(-max_count // 384))
    tt = -(-max_count // nt)
    tt = -(-tt // 4) * 4  # multiple of 4 for aligned fp32 rows
    return nt * tt, tt


def _pack_k128(a):
    """[K*128, F] -> [128, K*F]: partition-major packing of the SBUF layout."""
    k128, f = a.shape
    return np.ascontiguousarray(
        a.reshape(k128 // 128, 128, f).transpose(1, 0, 2).reshape(128, -1)
    )


def _prepare(x, Wg, bg, W1, b1, W2, b2):
    x = np.ascontiguousarray(np.asarray(x, dtype=np.float32))
    topk = _gating_topk(x, np.asarray(Wg, np.float32), np.asarray(bg, np.float32))
    idx = [np.nonzero((topk == e).any(axis=1))[0] for e in range(N_EXP)]
    counts = [len(i) for i in idx]
    cap, tt = _capacity(max(counts))
    nt = cap // tt

    bf16 = ml_dtypes.bfloat16
    in_maps = []
    for e in range(N_EXP):
        xg = np.zeros((cap, D_MODEL), np.float32)
        xg[: counts[e]] = x[idx[e]]
        xT = np.ascontiguousarray(xg.T).astype(bf16)  # [D, cap]
        xTp = _pack_k128(xT).reshape(128, KD, cap)  # [128, k, c]
        w1 = np.asarray(W1[e], np.float32).astype(bf16)  # [D, DFF]
        w1p = _pack_k128(w1).reshape(128, KD, D_FF)  # [128, k, f]
        w2 = np.asarray(W2[e], np.float32).astype(bf16)  # [DFF, D]
        w2p = _pack_k128(w2).reshape(128, KF, D_MODEL)  # [128, f, m]
        m = {
            "x0a": np.ascontiguousarray(xTp[:, :2, :tt]).reshape(128, -1),
            "x0b": np.ascontiguousarray(xTp[:, 2:5, :tt]).reshape(128, -1),
            "x0c": np.ascontiguousarray(xTp[:, 5:, :tt]).reshape(128, -1),
            "b1": np.ascontiguousarray(
                np.asarray(b1[e], np.float32).reshape(KF, 128).T
            ),
            "b2": np.ascontiguousarray(
                np.asarray(b2[e], np.float32).reshape(KD, 128).T
            ),
        }
        if nt > 1:
            m["x1"] = np.ascontiguousarray(xTp[:, :, tt:]).reshape(128, -1)
        for g, (lo, hi) in enumerate(W1_GROUPS):
            m[f"W1{g}"] = np.ascontiguousarray(w1p[:, :, lo:hi]).reshape(128, -1)
        for mi in range(KD):
            m[f"W2m{mi}"] = np.ascontiguousarray(
                w2p[:, :, mi * 128 : (mi + 1) * 128]
            ).reshape(128, -1)
        in_maps.append(m)
    return x, idx, counts, cap, tt, in_maps


def _run(x, Wg, bg, W1, b1, W2, b2, **run_kwargs):
    x, idx, counts, cap, tt, in_maps = _prepare(x, Wg, bg, W1, b1, W2, b2)
    key = (cap, tt)
    prog = _programs.get(key)
    if prog is None:
        prog = _programs.setdefault(key, _build_program(cap, tt))
    res = run_bass_kernel_spmd(
        prog, in_maps, core_ids=list(range(N_EXP)), **run_kwargs
    )
    out = np.zeros_like(x)
    for e in range(N_EXP):
        yp = np.asarray(res.results[e]["yT"], np.float32)  # [128, KD*cap]
        yT = yp.reshape(128, KD, cap).transpose(1, 0, 2).reshape(D_MODEL, cap)
        out[idx[e]] += yT[:, : counts[e]].T
    return out, res


def kernel(x, Wg, bg, W1, b1, W2, b2):
    out, _ = _run(x, Wg, bg, W1, b1, W2, b2)
    return out


# revision 18
# speedup vs baseline: 1.0490x; 1.0065x over previous
"""Expert-parallel MoE FFN for Trainium2 — one expert per NeuronCore (8 cores).

Strategy
--------
The reference computes, per token, the sum of top-2 expert FFN outputs (binary
combine mask, no gate weighting).  We shard along the expert axis: core ``e``
holds expert ``e``'s weights (W1[e], b1[e], W2[e], b2[e]) and processes only
the tokens that routed to it.

Host side (cheap, O(T*D*E) = 34 MFLOP):
  * gating softmax + top-2 (replicates jax.nn.softmax + jax.lax.top_k
    tie-breaking exactly: stable argsort on the fp32 scores, descending),
  * gather each expert's tokens, pad to a uniform capacity (all cores run the
    same NEFF), pre-transpose AND pre-pack every tensor into its exact SBUF
    layout ([128 partitions, flat free dim]) so each device DMA is a single
    trigger moving full-row (multi-KB) packets,
  * scatter-add the 8 per-expert outputs back into the [T, D] result.

Device side (the heavy part, ~18 GFLOP/core):
  hT = relu(W1^T-chained matmuls + b1);  yT = W2-chained matmuls + b2,
  everything kept in "transposed" layout: contraction dims live on SBUF
  partitions for both layers, so mm1's output feeds mm2 directly.
  bf16 inputs, fp32 PSUM accumulation, bf16 output.

Schedule (v2) — the warm matmul stream runs at the issue floor (~N/2.4GHz +
2.6ns), so the only recoverable time is at the edges:
  * ~56 zero-input warmup matmuls issue right after the engine preamble with
    no DMA dependency: they warm the PE HAM clock-gate (cold K=4/8 costs 2x)
    and absorb the DMA spin-up so real matmuls start warm and stall-free.
  * x token tiles ride the Sync/HWDGE queue concurrently with the W1 chain on
    the GpSimd/SWDGE queue (the two queues share the 16 DMA engines; measured
    aggregate ~360 GB/s).
  * W1 is split into ascending-size f-column groups so the first group lands
    fast; W2 is packed m-major (8 groups of [128, KF*128]) so groups complete
    in exactly mm2's m-loop consumption order.
  * y is emitted bf16 (halves the tail DMA; adds ~0.2% rms, well inside the
    2e-2 budget).
"""

import numpy as np
import ml_dtypes

import concourse.bacc as bacc
import concourse.mybir as mybir
import concourse.tile as tile
from concourse.bass_utils import run_bass_kernel_spmd
from concourse._compat import get_trn_type

D_MODEL = 1024
D_FF = 4096
N_EXP = 8
TOP_K = 2
KD = D_MODEL // 128  # 8 contraction chunks over d_model
KF = D_FF // 128  # 32 contraction chunks over d_ff

# W1 f-column groups: fine 128-col singles up front (whole-tile DMA deps —
# smaller groups complete sooner, so mm1's f-loop never outruns the ring),
# coarser groups once supply is comfortably ahead of consumption.
W1_GROUPS = [
    (0, 128), (128, 256), (256, 384), (384, 512),
    (512, 640), (640, 768), (768, 896), (896, 1024),
    (1024, 1536), (1536, 2176), (2176, 2944), (2944, 3712), (3712, D_FF),
]

# Zero-input warmup matmuls (N=128): ~107ns each cold; they bridge the gap
# between engine-preamble end (~6.6us) and the first operands landing
# (~11us) while warming the HAM clock gate (cold K=4/8 halves PE clock for
# ~3.4us after first activity; any PE idle gap >3.4us re-throttles).
N_WARM = 30

BF16 = mybir.dt.bfloat16
F32 = mybir.dt.float32

_programs: dict[tuple, object] = {}


def _build_program(cap: int, tt: int):
    """Bass/Tile program: pre-packed [D,cap] tokens -> 2-layer FFN -> output."""
    assert cap % tt == 0
    nt = cap // tt
    nc = bacc.Bacc(get_trn_type() or "TRN2", target_bir_lowering=False, debug=False)

    # x tile 0 in two k-halves (k0-3, k4-7) + the rest.
    xg_names = ["x0a", "x0b"] + (["x1"] if nt > 1 else [])
    xg_widths = [4 * tt, 4 * tt] + ([KD * (cap - tt)] if nt > 1 else [])
    xg_d = {
        n: nc.dram_tensor(n, [128, w], BF16, kind="ExternalInput").ap()
        for n, w in zip(xg_names, xg_widths)
    }
    w1_d = [
        nc.dram_tensor(f"W1{g}", [128, KD * (hi - lo)], BF16, kind="ExternalInput").ap()
        for g, (lo, hi) in enumerate(W1_GROUPS)
    ]
    # W2 m-major: group m holds [128 (f-part), KF * 128 m-cols].
    w2_d = [
        nc.dram_tensor(f"W2m{m}", [128, KF * 128], BF16, kind="ExternalInput").ap()
        for m in range(KD)
    ]
    b1_d = nc.dram_tensor("b1", [128, KF], F32, kind="ExternalInput").ap()
    b2_d = nc.dram_tensor("b2", [128, KD], F32, kind="ExternalInput").ap()
    y_d = nc.dram_tensor("yT", [128, KD * cap], BF16, kind="ExternalOutput").ap()
    y_v = y_d.rearrange("p (m c) -> p m c", c=cap)

    with tile.TileContext(nc) as tc:
        with (
            tc.tile_pool(name="sb", bufs=1) as sb,
            tc.tile_pool(name="hp", bufs=40) as hp,
            tc.tile_pool(name="yp", bufs=4) as yp,
            tc.tile_pool(name="pp1", bufs=6, space="PSUM") as pp1,
            tc.tile_pool(name="pp2", bufs=2, space="PSUM") as pp2,
        ):
            # ---- tiles ---------------------------------------------------
            x_sb = {
                n: sb.tile([128, d.shape[1]], BF16, tag=n, name=f"{n}_sb")
                for n, d in xg_d.items()
            }
            w1_tiles = [
                sb.tile([128, KD * (hi - lo)], BF16, tag=f"w1g{g}", name=f"w1g{g}")
                for g, (lo, hi) in enumerate(W1_GROUPS)
            ]
            w1_gs = [(lo, hi, t) for (lo, hi), t in zip(W1_GROUPS, w1_tiles)]
            b1_sb = sb.tile([128, KF], F32, tag="b1", name="b1_sb")
            b2_sb = sb.tile([128, KD], F32, tag="b2", name="b2_sb")
            w2_tiles = [
                sb.tile([128, KF * 128], BF16, tag=f"w2m{m}", name=f"w2m{m}")
                for m in range(KD)
            ]
            z_sb = sb.tile([128, 128], BF16, tag="zw", name="zw")

            # ---- input triggers + PE warmup ------------------------------
            # ALL inputs ride ONE queue (scalar — earliest preamble exit) in
            # exact consumption order: the 16-engine DMA pool round-robins
            # across active queues, so a second queue would steal half the
            # bandwidth from the critical prefix (measured: 3 active queues
            # cut the x+W1 prefix to ~95 GB/s and stalled the PE).
            nc.vector.memset(z_sb[:], 0.0)
            nc.scalar.dma_start(x_sb["x0a"][:], xg_d["x0a"])
            nc.scalar.dma_start(w1_tiles[0][:], w1_d[0])
            nc.scalar.dma_start(x_sb["x0b"][:], xg_d["x0b"])
            nc.scalar.dma_start(w1_tiles[1][:], w1_d[1])
            nc.scalar.dma_start(b1_sb[:], b1_d)
            for g in range(2, len(W1_GROUPS)):
                nc.scalar.dma_start(w1_tiles[g][:], w1_d[g])
            nc.scalar.dma_start(b2_sb[:], b2_d)
            for m in range(KD):
                nc.scalar.dma_start(w2_tiles[m][:], w2_d[m])
            if nt > 1:
                nc.scalar.dma_start(x_sb["x1"][:], xg_d["x1"])

            # Zero matmuls with no DMA dependency: keep the PE busy (and the
            # HAM clock-gate warming) while the first operands land.
            wps = pp2.tile([128, 128], F32, tag="ps2", name="warm_ps")
            for _ in range(N_WARM):
                nc.tensor.matmul(wps[:], z_sb[:], z_sb[:], start=True, stop=True)

            def x_rhs(k, it):
                if it == 0:
                    t = x_sb["x0a"] if k < 4 else x_sb["x0b"]
                    kk = k if k < 4 else k - 4
                    return t[:, kk * tt : (kk + 1) * tt]
                rest = cap - tt
                lo = k * rest + (it - 1) * tt
                return x_sb["x1"][:, lo : lo + tt]

            def w1_lhsT(k, f):
                col = f * 128
                for lo, hi, t in w1_gs:
                    if lo <= col < hi:
                        base = k * (hi - lo) + (col - lo)
                        return t[:, base : base + 128]
                raise AssertionError

            def w2_lhsT(f, m):
                return w2_tiles[m][:, f * 128 : (f + 1) * 128]

            # ---- compute --------------------------------------------------
            for it in range(nt):
                # mm1: hT[f*128+p, t] = relu(sum_d W1[d, f*128+p]*xT[d, t] + b1)
                h_tiles = []
                for f in range(KF):
                    ps = pp1.tile([128, tt], F32, tag="ps1", name=f"ps1_{it}_{f}")
                    for k in range(KD):
                        nc.tensor.matmul(
                            ps[:],
                            w1_lhsT(k, f),
                            x_rhs(k, it),
                            start=(k == 0),
                            stop=(k == KD - 1),
                        )
                    # relu on the VECTOR engine: the scalar engine spends the
                    # first ~23us issuing the 26 serialized DMA triggers.
                    ht = hp.tile([128, tt], BF16, tag="h", name=f"h_{it}_{f}")
                    nc.vector.tensor_scalar(
                        ht[:],
                        ps[:],
                        b1_sb[:, f : f + 1],
                        0.0,
                        mybir.AluOpType.add,
                        mybir.AluOpType.max,
                    )
                    h_tiles.append(ht)

                # mm2: yT[m*128+p, t] = sum_f W2[f, m*128+p] * hT[f, t] + b2
                for m in range(KD):
                    ps2 = pp2.tile([128, tt], F32, tag="ps2", name=f"ps2_{it}_{m}")
                    for f in range(KF):
                        nc.tensor.matmul(
                            ps2[:],
                            w2_lhsT(f, m),
                            h_tiles[f][:],
                            start=(f == 0),
                            stop=(f == KF - 1),
                        )
                    last = it == nt - 1 and m == KD - 1
                    yt = yp.tile([128, tt], BF16, tag="y", name=f"y_{it}_{m}")
                    if not last:
                        nc.vector.tensor_scalar_add(
                            yt[:], ps2[:], b2_sb[:, m : m + 1]
                        )
                        nc.sync.dma_start(y_v[:, m, it * tt : (it + 1) * tt], yt[:])
                    else:
                        # Critical tail: the add runs on the scalar engine
                        # (idle at the end; the vector engine still drains the
                        # previous m's add when the last matmul completes).
                        nc.scalar.activation(
                            yt[:],
                            ps2[:],
                            mybir.ActivationFunctionType.Identity,
                            bias=b2_sb[:, m : m + 1],
                        )
                        nc.sync.dma_start(y_v[:, m, it * tt : (it + 1) * tt], yt[:])

    nc.compile()
    return nc


def _gating_topk(x, Wg, bg):
    """Replicates jax.nn.softmax + jax.lax.top_k(..., 2) in fp32 numpy."""
    logits = x @ Wg + bg
    m = logits.max(axis=1, keepdims=True)
    e = np.exp(logits - m)
    scores = e / e.sum(axis=1, keepdims=True)
    # top_k: descending, ties broken toward the lower index (stable).
    order = np.argsort(-scores, axis=1, kind="stable")
    return order[:, :TOP_K]


def _capacity(max_count):
    # Token tile <= 384: keeps one fp32 PSUM bank per matmul (<=512) AND the
    # resident-weights SBUF budget valid for capacities well beyond the
    # ~1024+-27 expert loads this distribution produces.
    nt = max(1, -(-max_count // 384))
    tt = -(-max_count // nt)
    tt = -(-tt // 4) * 4  # multiple of 4 for aligned fp32 rows
    return nt * tt, tt


def _pack_k128(a):
    """[K*128, F] -> [128, K*F]: partition-major packing of the SBUF layout."""
    k128, f = a.shape
    return np.ascontiguousarray(
        a.reshape(k128 // 128, 128, f).transpose(1, 0, 2).reshape(128, -1)
    )


def _prepare(x, Wg, bg, W1, b1, W2, b2):
    x = np.ascontiguousarray(np.asarray(x, dtype=np.float32))
    topk = _gating_topk(x, np.asarray(Wg, np.float32), np.asarray(bg, np.float32))
    idx = [np.nonzero((topk == e).any(axis=1))[0] for e in range(N_EXP)]
    counts = [len(i) for i in idx]
    cap, tt = _capacity(max(counts))
    nt = cap // tt

    bf16 = ml_dtypes.bfloat16
    in_maps = []
    for e in range(N_EXP):
        xg = np.zeros((cap, D_MODEL), np.float32)
        xg[: counts[e]] = x[idx[e]]
        xT = np.ascontiguousarray(xg.T).astype(bf16)  # [D, cap]
        xTp = _pack_k128(xT).reshape(128, KD, cap)  # [128, k, c]
        w1 = np.asarray(W1[e], np.float32).astype(bf16)  # [D, DFF]
        w1p = _pack_k128(w1).reshape(128, KD, D_FF)  # [128, k, f]
        w2 = np.asarray(W2[e], np.float32).astype(bf16)  # [DFF, D]
        w2p = _pack_k128(w2).reshape(128, KF, D_MODEL)  # [128, f, m]
        m = {
            "x0a": np.ascontiguousarray(xTp[:, :4, :tt]).reshape(128, -1),
            "x0b": np.ascontiguousarray(xTp[:, 4:, :tt]).reshape(128, -1),
            "b1": np.ascontiguousarray(
                np.asarray(b1[e], np.float32).reshape(KF, 128).T
            ),
            "b2": np.ascontiguousarray(
                np.asarray(b2[e], np.float32).reshape(KD, 128).T
            ),
        }
        if nt > 1:
            m["x1"] = np.ascontiguousarray(xTp[:, :, tt:]).reshape(128, -1)
        for g, (lo, hi) in enumerate(W1_GROUPS):
            m[f"W1{g}"] = np.ascontiguousarray(w1p[:, :, lo:hi]).reshape(128, -1)
        for mi in range(KD):
            m[f"W2m{mi}"] = np.ascontiguousarray(
                w2p[:, :, mi * 128 : (mi + 1) * 128]
            ).reshape(128, -1)
        in_maps.append(m)
    return x, idx, counts, cap, tt, in_maps


def _run(x, Wg, bg, W1, b1, W2, b2, **run_kwargs):
    x, idx, counts, cap, tt, in_maps = _prepare(x, Wg, bg, W1, b1, W2, b2)
    key = (cap, tt)
    prog = _programs.get(key)
    if prog is None:
        prog = _programs.setdefault(key, _build_program(cap, tt))
    res = run_bass_kernel_spmd(
        prog, in_maps, core_ids=list(range(N_EXP)), **run_kwargs
    )
    out = np.zeros_like(x)
    for e in range(N_EXP):
        yp = np.asarray(res.results[e]["yT"], np.float32)  # [128, KD*cap]
        yT = yp.reshape(128, KD, cap).transpose(1, 0, 2).reshape(D_MODEL, cap)
        out[idx[e]] += yT[:, : counts[e]].T
    return out, res


def kernel(x, Wg, bg, W1, b1, W2, b2):
    out, _ = _run(x, Wg, bg, W1, b1, W2, b2)
    return out
